# revision 2
# baseline (speedup 1.0000x reference)
"""Depthwise 4x4 separable blur (upfirdn2d pad=(2,1)) on 8 TRN2 NeuronCores.

Strategy (pure data parallel over batch, B=8 -> 1 image per core):
  - Per core image: [C=128, H=256, W=256] fp32. Partition dim = h (2 blocks
    of 128 rows), free dim = (channel-group, w).
  - W-pass on VectorE: the separable 4-tap row conv collapses to two
    scalar_tensor_tensor ops per tile: t1 = (x[w-2]*r1 + x[w+1]),
    t2 = (x[w-1]*r2 + x[w]), computed on a 259-wide zero-padded row layout.
    Outputs are rounded to float32r (required by the fast PE matmul path).
  - H-pass on TensorE: column conv as banded matmuls accumulated in PSUM:
    out = (kw3*A)^T t1 + (kw2*A)^T t2 (+ cross-block seam tails, expressed
    as full-K matmuls whose weight matrices are zero except at the 1-2 halo
    rows, so every operand stays at base_partition 0).
  - ScalarE copies PSUM -> SBUF, DMA writes back to HBM.
"""

import os
import sys

import numpy as np

for _p in ("/opt/trn_rl_repo", "/root/.axon_site/_ro/trn_rl_repo"):
    if os.path.isdir(_p) and _p not in sys.path:
        sys.path.append(_p)

import concourse.bacc as bacc
import concourse.mybir as mybir
from concourse import tile
from concourse.bass_utils import run_bass_kernel_spmd

B, C, H, W = 8, 128, 256, 256
N_CORES = 8
CG = 8               # channels per inner tile group
NG = C // CG         # 16 groups
WP = W + 3           # padded row width (2 left + 1 right zero pad)
FG = CG * W          # free-dim elements per t-tile
NPAIR = CG // 2      # channel pairs per group (F=512 per matmul)
KS = 4               # conv kernel size


def _build_bands(kern: np.ndarray):
    """Factor the (flipped) 2D kernel into kh (x) kw and build the six
    [128,128] banded lhsT matrices for the H-pass matmuls."""
    k = np.flip(kern.astype(np.float64), (0, 1))
    u, s, vt = np.linalg.svd(k)
    assert s[1] < 1e-6 * s[0], "blur kernel must be separable"
    kh = u[:, 0] * np.sqrt(s[0])
    kw = vt[0] * np.sqrt(s[0])
    if kh.sum() < 0:
        kh, kw = -kh, -kw
    assert np.allclose(np.outer(kh, kw), k, atol=1e-12 + 1e-7 * np.abs(k).max())
    assert abs(kw[3]) > 1e-12 and abs(kw[2]) > 1e-12
    r1 = float(kw[0] / kw[3])
    r2 = float(kw[1] / kw[2])

    # Main band: out row m takes input rows i = m + t - 2 (t = tap 0..3).
    A = np.zeros((128, 128), np.float64)
    for m in range(128):
        for t in range(KS):
            i = m + t - 2
            if 0 <= i < 128:
                A[i, m] = kh[t]
    # Block-0 tail: inputs are block-1 partitions p (global row 128+p).
    B0 = np.zeros((128, 128), np.float64)
    for p in range(128):
        for m in range(128):
            t = (128 + p) - m + 2
            if 0 <= t < KS:
                B0[p, m] = kh[t]
    # Block-1 tail: inputs are block-0 partitions p (global row p).
    B1 = np.zeros((128, 128), np.float64)
    for p in range(128):
        for mp in range(128):
            t = p - (128 + mp) + 2
            if 0 <= t < KS:
                B1[p, mp] = kh[t]
    bands = np.stack(
        [A * kw[3], A * kw[2], B0 * kw[3], B0 * kw[2], B1 * kw[3], B1 * kw[2]]
    ).astype(np.float32)
    return bands, r1, r2


def _build_nc(r1: float, r2: float):
    nc = bacc.Bacc("TRN2", target_bir_lowering=False, debug=False,
                   num_devices=N_CORES)
    x = nc.dram_tensor("input", [C, H, W], mybir.dt.float32,
                       kind="ExternalInput").ap()
    bands = nc.dram_tensor("bands", [6, 128, 128], mybir.dt.float32,
                           kind="ExternalInput").ap()
    out = nc.dram_tensor("output", [C, H, W], mybir.dt.float32,
                         kind="ExternalOutput").ap()
    mult = mybir.AluOpType.mult
    add = mybir.AluOpType.add

    with tile.TileContext(nc) as tc:
        with (
            tc.tile_pool(name="bands", bufs=1) as bp,
            tc.tile_pool(name="xp", bufs=2) as xpp,
            tc.tile_pool(name="tp", bufs=2) as tp,
            tc.tile_pool(name="osb", bufs=6) as osb,
            tc.tile_pool(name="ps", bufs=4, space="PSUM") as pp,
        ):
            wmats = []
            for q in range(6):
                bt = bp.tile([128, 128], mybir.dt.float32, tag=f"bf{q}")
                nc.sync.dma_start(bt[:], bands[q])
                br = bp.tile([128, 128], mybir.dt.float32r, tag=f"br{q}")
                nc.vector.tensor_copy(br[:], bt[:])
                wmats.append(br)
            A1, A2, B01, B02, B11, B12 = wmats

            for g in range(NG):
                c0 = g * CG
                t1 = {}
                t2 = {}
                for b in (0, 1):
                    h0 = b * 128
                    xp = xpp.tile([128, CG * WP], mybir.dt.float32, tag=f"x{b}")
                    xr = xp[:].rearrange("p (c w) -> p c w", c=CG)
                    nc.gpsimd.memset(xr[:, :, 0:2], 0.0)
                    nc.gpsimd.memset(xr[:, :, W + 2:W + 3], 0.0)
                    nc.sync.dma_start(
                        xr[:, :, 2:W + 2],
                        x[c0:c0 + CG, h0:h0 + 128, :].transpose([1, 0, 2]),
                    )
                    t1t = tp.tile([128, FG], mybir.dt.float32r, tag=f"t1{b}")
                    t2t = tp.tile([128, FG], mybir.dt.float32r, tag=f"t2{b}")
                    t1v = t1t[:].rearrange("p (c w) -> p c w", c=CG)
                    t2v = t2t[:].rearrange("p (c w) -> p c w", c=CG)
                    nc.vector.scalar_tensor_tensor(
                        t1v, xr[:, :, 0:W], r1, xr[:, :, 3:W + 3], mult, add)
                    nc.vector.scalar_tensor_tensor(
                        t2v, xr[:, :, 1:W + 1], r2, xr[:, :, 2:W + 2], mult, add)
                    t1[b] = t1t
                    t2[b] = t2t

                for b in (0, 1):
                    tw1, tw2 = (B01, B02) if b == 0 else (B11, B12)
                    ob = 1 - b
                    for pr in range(NPAIR):
                        fs = slice(pr * 512, (pr + 1) * 512)
                        ps = pp.tile([128, 512], mybir.dt.float32, tag="ps")
                        nc.tensor.matmul(ps[:], A1[:], t1[b][:, fs],
                                         start=True, stop=False)
                        nc.tensor.matmul(ps[:], A2[:], t2[b][:, fs],
                                         start=False, stop=False)
                        nc.tensor.matmul(ps[:], tw1[:], t1[ob][:, fs],
                                         start=False, stop=False)
                        nc.tensor.matmul(ps[:], tw2[:], t2[ob][:, fs],
                                         start=False, stop=True)
                        ot = osb.tile([128, 512], mybir.dt.float32, tag="o")
                        nc.scalar.copy(ot[:], ps[:])
                        co = c0 + pr * 2
                        nc.sync.dma_start(
                            out[co:co + 2, b * 128:(b + 1) * 128, :]
                            .transpose([1, 0, 2]),
                            ot[:].rearrange("p (c w) -> p c w", c=2),
                        )
    nc.compile()
    return nc


_CACHE = {}


def _get_nc(r1: float, r2: float):
    key = (r1, r2)
    if key not in _CACHE:
        _CACHE[key] = _build_nc(r1, r2)
    return _CACHE[key]


def kernel(**inputs) -> np.ndarray:
    x = np.asarray(inputs["input"], dtype=np.float32)
    kern = np.asarray(inputs["kernel"], dtype=np.float32)
    assert x.shape == (B, C, H, W) and kern.shape == (KS, KS)
    bands, r1, r2 = _build_bands(kern)
    nc = _get_nc(r1, r2)
    in_maps = [
        {"input": np.ascontiguousarray(x[i]), "bands": bands}
        for i in range(N_CORES)
    ]
    res = run_bass_kernel_spmd(nc, in_maps, list(range(N_CORES)))
    global _LAST_RESULTS
    _LAST_RESULTS = res
    return np.stack([res.results[i]["output"] for i in range(N_CORES)])


if __name__ == "__main__":
    rng = np.random.default_rng(0)
    x = rng.standard_normal((B, C, H, W), dtype=np.float32)
    k1 = np.array([1.0, 3.0, 3.0, 1.0], np.float64)
    k = np.outer(k1, k1)
    k = (k / k.sum() * 4).astype(np.float32)
    y = kernel(input=x, kernel=k)
    print("out", y.shape, y.dtype, float(np.abs(y).max()))
